# revision 16
# baseline (speedup 1.0000x reference)
"""Causal self-attention kernel for 8 Trainium2 NeuronCores — v2.

Problem: y = CausalSelfAttention(x) with B=2, T=2048, C=2048, 16 heads,
head_dim=128, fp32 in/out (bf16 internally; rel-err budget 2e-2).

Sharding (8 cores): core = (b, hg), b in {0,1} batch, hg in {0..3} head
group of 4 heads.  Each core computes QKV for its heads, attention, and a
partial c_proj; host sums the 4 partials per batch.

v2 structure (single fused pass, software-scheduled interleave):
  - x^T is streamed ONCE; Q,K,V projections computed per 512-col time
    block (tt) with h-major PSUM rotation (boot block tt=0 is ci-major so
    the PE ramps at DMA delivery speed).
  - attention for q-block qt runs interleaved with QKV of block tt=qt+1
    and c_proj of block qt-2, merged by a virtual-PE-time scheduler, so
    the scalar engine's exp work hides under PE matmuls.
  - attention processes heads in lockstep pairs with the AV matmul
    deferred one k-step, so the in-order PE queue never parks behind the
    exp it just issued.
  - softmax denominator: DVE accumulates sum of exp tiles (ea += e), one
    ones-matmul per (head, qt) reduces across partitions, fast-approx
    reciprocal, broadcast multiply.
Everything PE-facing is bf16 (1 elem/cycle like fp32r, but LDWEIGHTS and
DMA are 2x faster); PSUM accumulation fp32; output partials bf16.
"""

import numpy as np

import concourse.bass as bass
import concourse.mybir as mybir
import concourse.tile as tile
from concourse.bass_utils import run_bass_kernel_spmd

B, T, C = 2, 2048, 2048
N_HEAD = 16
HD = 128
HPC = 4          # heads per core
HCOLS = HPC * HD  # 512 columns of W per core per projection
P = 128          # partitions
QT = 512         # q-block (free dim) for projections and attention
KT = 128         # k-tile in attention
NQT = T // QT    # 4
NCT = C // P     # 16 contraction tiles for projections
SCALE = 1.0 / np.sqrt(HD)

F32 = mybir.dt.float32
F32R = mybir.dt.float32r
BF16 = mybir.dt.bfloat16
MMDT = BF16


def build_nc(split_waits=True):
    nc = bass.Bass("TRN2", target_bir_lowering=False, debug=False)

    xT = nc.dram_tensor("xT", [C, T], MMDT, kind="ExternalInput").ap()
    wq = nc.dram_tensor("wq", [C, HCOLS], MMDT, kind="ExternalInput").ap()
    wk = nc.dram_tensor("wk", [C, HCOLS], MMDT, kind="ExternalInput").ap()
    wv = nc.dram_tensor("wv", [C, HCOLS], MMDT, kind="ExternalInput").ap()
    bq = nc.dram_tensor("bq", [P, HPC], F32, kind="ExternalInput").ap()
    bk = nc.dram_tensor("bk", [P, HPC], F32, kind="ExternalInput").ap()
    bv = nc.dram_tensor("bv", [P, HCOLS], F32, kind="ExternalInput").ap()
    wp = nc.dram_tensor("wp", [HCOLS, C], MMDT, kind="ExternalInput").ap()
    # negm[r, c]: -1e30 where r > c-512 else 0.  The window
    # negm[:, 512-128j : 640-128j] is the additive causal mask for the
    # j-th diagonal k-tile of a 512-wide q-block (full -inf for the j
    # leading 128-col groups, lower-triangle on the diagonal group).
    negm = nc.dram_tensor("negm", [P, QT + P], F32, kind="ExternalInput").ap()
    ones = nc.dram_tensor("ones", [P, P], MMDT, kind="ExternalInput").ap()
    out = nc.dram_tensor("out", [T, C], MMDT, kind="ExternalOutput").ap()

    with tile.TileContext(nc) as tc:
        _build(tc, xT, wq, wk, wv, bq, bk, bv, wp, negm, ones, out)
    if split_waits:
        _split_matmul_waits(nc)
    return nc


def _split_matmul_waits(nc):
    """Lowered instructions fit only ONE sync-wait command (walrus: 'Too many
    sync wait commands').  Move excess waits onto preceding same-engine
    EventSemaphore instructions (which hold 2 waits each)."""
    n = 0
    for f in nc.m.functions:
        for b in f.blocks:
            patched = []
            changed = False
            for inst in b.instructions:
                si = inst.sync_info
                if (
                    not isinstance(inst, mybir.InstEventSemaphore)
                    and si is not None
                    and len(si.on_wait) > 1
                ):
                    waits = list(si.on_wait)
                    extra, keep = waits[:-1], waits[-1:]
                    for ci in range(0, len(extra), 2):
                        n += 1
                        patched.append(
                            mybir.InstEventSemaphore(
                                name=f"{inst.name}-wsplit{ci}",
                                engine=inst.engine,
                                ins=[],
                                outs=[],
                                sync_info=mybir.SyncInfo(
                                    on_wait=extra[ci:ci + 2], on_update=[]
                                ),
                            )
                        )
                    si.on_wait = keep
                    changed = True
                patched.append(inst)
            if changed:
                b.instructions = patched
    return n


def _merge(streams):
    """Emit thunks from several (cost, fn) lists, interleaved so each
    stream's cumulative-cost fraction advances evenly (virtual PE time)."""
    seqs = [s for s in streams if s]
    totals = [sum(c for c, _ in s) for s in seqs]
    pos = [0] * len(seqs)
    spent = [0.0] * len(seqs)
    while True:
        live = [i for i in range(len(seqs)) if pos[i] < len(seqs[i])]
        if not live:
            break
        i = min(live, key=lambda j: spent[j] / totals[j])
        c, fn = seqs[i][pos[i]]
        fn()
        spent[i] += c
        pos[i] += 1


def _build(tc, xT, wq, wk, wv, bq, bk, bv, wp, negm, ones, out):
    from contextlib import ExitStack

    nc = tc.nc
    Exp = mybir.ActivationFunctionType.Exp
    Ln = mybir.ActivationFunctionType.Ln
    Ident = mybir.ActivationFunctionType.Identity

    with ExitStack() as root:
        # ---- SBUF residents ----------------------------------------------
        res = root.enter_context(tc.tile_pool(name="res", bufs=1))
        qt_sb = res.tile([P, HPC, T], MMDT, tag="qt_sb")
        kt_sb = res.tile([P, HPC, T], MMDT, tag="kt_sb")
        v_sb = res.tile([P, NCT, HCOLS], MMDT, tag="v_sb")
        yt_sb = res.tile([P, HPC, T], MMDT, tag="yt_sb")
        wq_sb = res.tile([P, NCT, HCOLS], MMDT, tag="wq_sb")
        wk_sb = res.tile([P, NCT, HCOLS], MMDT, tag="wk_sb")
        wv_sb = res.tile([P, NCT, HCOLS], MMDT, tag="wv_sb")
        wp_sb = res.tile([P, HPC, C], MMDT, tag="wp_sb")
        bq_sb = res.tile([P, HPC], F32, tag="bq_sb")
        bk_sb = res.tile([P, HPC], F32, tag="bk_sb")
        bv_sb = res.tile([P, HCOLS], F32, tag="bv_sb")
        negm_sb = res.tile([P, QT + P], F32, tag="negm_sb")
        ones_sb = res.tile([P, P], MMDT, tag="ones_sb")

        xpool = root.enter_context(tc.tile_pool(name="xpool", bufs=2))
        epool = root.enter_context(tc.tile_pool(name="epool", bufs=4))
        eapool = root.enter_context(tc.tile_pool(name="eapool", bufs=2))
        rpool = root.enter_context(tc.tile_pool(name="rpool", bufs=2))
        opool = root.enter_context(tc.tile_pool(name="opool", bufs=2))

        # ---- input DMAs ---------------------------------------------------
        # sync queue: biases, then weight slabs in ci order (consumption
        # order of the ci-major boot block).  gpsimd queue: x tiles in ci
        # order, then masks/ones, then the tt=1 x prefetch.
        nc.sync.dma_start(out=bq_sb[:, :], in_=bq)
        nc.sync.dma_start(out=bk_sb[:, :], in_=bk)
        wqr = wq.rearrange("(co ci) n -> ci co n", ci=P)
        wkr = wk.rearrange("(co ci) n -> ci co n", ci=P)
        wvr = wv.rearrange("(co ci) n -> ci co n", ci=P)
        wpr = wp.rearrange("(ht p) c -> p ht c", p=P)

        xts = {}
        xTr = xT.rearrange("(ci p) t -> p ci t", p=P)

        def xt_load(tt, fine=False):
            t = xpool.tile([P, NCT, QT], MMDT, tag="xt", name=f"xt{tt}")
            if fine:
                # boot block consumes ci-by-ci as the DMA delivers
                for ci in range(NCT):
                    nc.gpsimd.dma_start(
                        out=t[:, ci, :],
                        in_=xTr[:, ci, tt * QT:(tt + 1) * QT],
                    )
            else:
                nc.gpsimd.dma_start(
                    out=t[:, :, :], in_=xTr[:, :, tt * QT:(tt + 1) * QT]
                )
            xts[tt] = t

        xt_load(0, fine=True)
        for ci in range(NCT):
            nc.sync.dma_start(out=wq_sb[:, ci, :], in_=wqr[:, ci, :])
        nc.gpsimd.dma_start(out=negm_sb[:, :], in_=negm)
        nc.gpsimd.dma_start(out=ones_sb[:, :], in_=ones)
        xt_load(1)
        for ci in range(NCT):
            nc.sync.dma_start(out=wk_sb[:, ci, :], in_=wkr[:, ci, :])
        nc.sync.dma_start(out=bv_sb[:, :], in_=bv)
        for ci in range(NCT):
            nc.sync.dma_start(out=wv_sb[:, ci, :], in_=wvr[:, ci, :])
        for i in range(HPC):
            nc.sync.dma_start(out=wp_sb[:, i, :], in_=wpr[:, i, :])

        # ---- boot block: QKV for tt=0, ci-major (consume DMA in order) ---
        with ExitStack() as bootctx:
            boot_psum = bootctx.enter_context(
                tc.tile_pool(name="boot_psum", bufs=6, space="PSUM")
            )
            xt0 = xts[0]
            for kind, w_sb, drain in (
                ("q", wq_sb, "q"), ("k", wk_sb, "k"), ("v", wv_sb, "v"),
            ):
                ps = [
                    boot_psum.tile([P, QT], F32, tag="boot", name=f"b{kind}{i}")
                    for i in range(4)
                ]
                for ci in range(NCT):
                    for i in range(4):
                        if kind == "v":
                            lhsT = xt0[:, ci, i * P:(i + 1) * P]
                            rhs = wv_sb[:, ci, :]
                        else:
                            lhsT = w_sb[:, ci, i * HD:(i + 1) * HD]
                            rhs = xt0[:, ci, :]
                        nc.tensor.matmul(
                            ps[i][:, :], lhsT, rhs,
                            start=(ci == 0), stop=(ci == NCT - 1),
                        )
                for i in range(4):
                    if kind == "q":
                        nc.scalar.activation(
                            qt_sb[:, i, 0:QT], ps[i][:, :], Ident,
                            bias=bq_sb[:, i:i + 1],
                        )
                    elif kind == "k":
                        nc.scalar.activation(
                            kt_sb[:, i, 0:QT], ps[i][:, :], Ident,
                            bias=bk_sb[:, i:i + 1],
                        )
                    else:
                        nc.vector.tensor_add(
                            v_sb[:, i, :], ps[i][:, :], bv_sb[:, :]
                        )

        # ---- main pools ---------------------------------------------------
        pq_psum = root.enter_context(tc.tile_pool(name="pq_psum", bufs=2, space="PSUM"))
        s_psum = root.enter_context(tc.tile_pool(name="s_psum", bufs=2, space="PSUM"))
        y_psum = root.enter_context(tc.tile_pool(name="y_psum", bufs=2, space="PSUM"))
        d_psum = root.enter_context(tc.tile_pool(name="d_psum", bufs=2, space="PSUM"))

        # ---- stream builders ---------------------------------------------
        def qkv_stream(tt):
            """QKV projections for block tt, h-major (weights resident)."""
            th = []
            if tt + 1 < NQT:
                th.append((0.0, lambda tt=tt: xt_load(tt + 1)))
            state = {}

            def quad(kind, i, cq, tt=tt):
                def fn(kind=kind, i=i, cq=cq, tt=tt):
                    if cq == 0:
                        state[(kind, i)] = pq_psum.tile(
                            [P, QT], F32, tag="pq", name=f"p{kind}{tt}_{i}"
                        )
                    ps = state[(kind, i)]
                    xt_t = xts[tt]
                    for ci in range(cq * 4, cq * 4 + 4):
                        if kind == "v":
                            lhsT = xt_t[:, ci, i * P:(i + 1) * P]
                            rhs = wv_sb[:, ci, :]
                        elif kind == "q":
                            lhsT = wq_sb[:, ci, i * HD:(i + 1) * HD]
                            rhs = xt_t[:, ci, :]
                        else:
                            lhsT = wk_sb[:, ci, i * HD:(i + 1) * HD]
                            rhs = xt_t[:, ci, :]
                        nc.tensor.matmul(
                            ps[:, :], lhsT, rhs,
                            start=(ci == 0), stop=(ci == NCT - 1),
                        )
                    if cq == 3:
                        if kind == "q":
                            nc.scalar.activation(
                                qt_sb[:, i, tt * QT:(tt + 1) * QT], ps[:, :],
                                Ident, bias=bq_sb[:, i:i + 1],
                            )
                        elif kind == "k":
                            nc.scalar.activation(
                                kt_sb[:, i, tt * QT:(tt + 1) * QT], ps[:, :],
                                Ident, bias=bk_sb[:, i:i + 1],
                            )
                        else:
                            nc.vector.tensor_add(
                                v_sb[:, tt * 4 + i, :], ps[:, :], bv_sb[:, :]
                            )
                return fn

            for kind in ("q", "k", "v"):
                for i in range(4):
                    for cq in range(4):
                        th.append((852.0, quad(kind, i, cq)))
            return th

        def attn_stream(qt):
            """Attention for q-block qt: head pairs in lockstep, AV matmul
            deferred one k-step behind the S matmul + exp.  Causal mask is
            added to S in PSUM (windowed -1e30 const) BEFORE exp, so the
            exp output feeds the AV matmul with no DVE op in between."""
            nkt = 4 * qt + 4
            th = []
            for hp in range(2):
                h0 = 2 * hp
                st = {}

                def stepA(h, kt, qt=qt, st=st):
                    def fn(h=h, kt=kt, qt=qt, st=st):
                        if kt == 0:
                            st[("y", h)] = y_psum.tile(
                                [P, QT], F32, tag="y", name=f"y{h}_{qt}"
                            )
                            st[("ea", h)] = eapool.tile(
                                [P, QT], MMDT, tag="ea", name=f"ea{h}_{qt}"
                            )
                        # diagonal k-tiles: columns [0, j*128) are fully
                        # masked — compute only the live window [w0, QT)
                        j = kt - 4 * qt
                        w0 = max(j, 0) * P
                        s_ps = s_psum.tile([P, QT], F32, tag="s", name=f"s{h}_{kt}")
                        nc.tensor.matmul(
                            s_ps[:, w0:QT],
                            kt_sb[:, h, kt * KT:(kt + 1) * KT],
                            qt_sb[:, h, qt * QT + w0:(qt + 1) * QT],
                            start=True, stop=True,
                        )
                        if j >= 0:
                            nc.vector.tensor_add(
                                s_ps[:, w0:w0 + P], s_ps[:, w0:w0 + P],
                                negm_sb[:, QT:QT + P],
                            )
                        e_sb = epool.tile([P, QT], MMDT, tag="e", name=f"e{h}_{kt}")
                        nc.scalar.activation(
                            e_sb[:, w0:QT], s_ps[:, w0:QT], Exp, scale=float(SCALE)
                        )
                        st[("e", h, kt)] = (e_sb, w0)
                    return fn

                def stepB(h, kt, qt=qt, st=st, nkt=nkt):
                    def fn(h=h, kt=kt, qt=qt, st=st, nkt=nkt):
                        e_sb, w0 = st.pop(("e", h, kt))
                        ea_sb = st[("ea", h)]
                        if kt == 0:
                            nc.vector.tensor_copy(ea_sb[:, :], e_sb[:, :])
                        else:
                            nc.vector.tensor_add(
                                ea_sb[:, w0:QT], ea_sb[:, w0:QT], e_sb[:, w0:QT]
                            )
                        y_ps = st[("y", h)]
                        nc.tensor.matmul(
                            y_ps[:, w0:QT],
                            v_sb[:, kt, h * HD:(h + 1) * HD],
                            e_sb[:, w0:QT],
                            start=(kt == 0), stop=(kt == nkt - 1),
                            skip_group_check=(w0 > 0),
                        )
                        if kt == nkt - 1:
                            den_ps = d_psum.tile(
                                [P, QT], F32, tag="den", name=f"den{h}_{qt}"
                            )
                            nc.tensor.matmul(
                                den_ps[:, :], ones_sb[:, :], ea_sb[:, :],
                                start=True, stop=True,
                            )
                            # 1/den = exp(-ln(den)) on ACT (ln and exp share
                            # the natural_log_exp_and_others table set) —
                            # ~5x cheaper than DVE's iterative reciprocal
                            rln = rpool.tile(
                                [P, QT], F32, tag="rln", name=f"rln{h}_{qt}"
                            )
                            nc.scalar.activation(rln[:, :], den_ps[:, :], Ln)
                            rbc = rpool.tile(
                                [P, QT], F32, tag="rbc", name=f"rbc{h}_{qt}"
                            )
                            nc.scalar.activation(
                                rbc[:, :], rln[:, :], Exp, scale=-1.0
                            )
                            nc.vector.tensor_mul(
                                yt_sb[:, h, qt * QT:(qt + 1) * QT],
                                y_ps[:, :], rbc[:, :],
                            )
                    return fn

                # lockstep pair, B deferred one k-step
                for kt in range(nkt):
                    th.append((213.0, stepA(h0, kt)))
                    th.append((213.0, stepA(h0 + 1, kt)))
                    if kt > 0:
                        th.append((213.0, stepB(h0, kt - 1)))
                        th.append((213.0, stepB(h0 + 1, kt - 1)))
                th.append((426.0, stepB(h0, nkt - 1)))
                th.append((426.0, stepB(h0 + 1, nkt - 1)))
            return th

        def proj_stream(qt):
            """Partial c_proj for the four 128-row tiles of q-block qt."""
            th = []
            state = {}

            def tile_fn(qi, ct):
                def fn(qi=qi, ct=ct):
                    if ct == 0:
                        state[qi] = opool.tile([P, C], MMDT, tag="o", name=f"o{qi}")
                    o_sb = state[qi]
                    cp = pq_psum.tile([P, QT], F32, tag="pq", name=f"cp{qi}_{ct}")
                    for h in range(HPC):
                        nc.tensor.matmul(
                            cp[:, :],
                            yt_sb[:, h, qi * P:(qi + 1) * P],
                            wp_sb[:, h, ct * QT:(ct + 1) * QT],
                            start=(h == 0), stop=(h == HPC - 1),
                        )
                    nc.vector.tensor_copy(o_sb[:, ct * QT:(ct + 1) * QT], cp[:, :])
                    nc.sync.dma_start(
                        out=out[qi * P:(qi + 1) * P, ct * QT:(ct + 1) * QT],
                        in_=o_sb[:, ct * QT:(ct + 1) * QT],
                    )
                return fn

            for qi in range(4 * qt, 4 * qt + 4):
                for ct in range(4):
                    th.append((852.0, tile_fn(qi, ct)))
            return th

        # ---- schedule -----------------------------------------------------
        _merge([qkv_stream(1), attn_stream(0)])
        _merge([qkv_stream(2), attn_stream(1), proj_stream(0)])
        _merge([qkv_stream(3), attn_stream(2), proj_stream(1)])
        _merge([attn_stream(3), proj_stream(2)])
        _merge([proj_stream(3)])


def make_core_inputs(x, W_attn, b_attn, W_proj, b_proj):
    """Host-side shard/prep. Returns list of 8 input dicts."""
    import ml_dtypes

    bf16 = ml_dtypes.bfloat16
    x = np.asarray(x, dtype=np.float32)
    W_attn = np.asarray(W_attn, dtype=np.float32)
    b_attn = np.asarray(b_attn, dtype=np.float32)
    W_proj = np.asarray(W_proj, dtype=np.float32)
    b_proj = np.asarray(b_proj, dtype=np.float32)

    r = np.arange(P)[:, None]
    c = np.arange(QT + P)[None, :]
    negm = np.where(r > c - QT, np.float32(-1e30), np.float32(0.0))
    negm = np.ascontiguousarray(negm.astype(np.float32))

    in_maps = []
    for core in range(8):
        b, hg = divmod(core, 4)
        cs = slice(HCOLS * hg, HCOLS * hg + HCOLS)
        in_maps.append(
            {
                "xT": np.ascontiguousarray(x[b].T).astype(bf16),
                "wq": np.ascontiguousarray(W_attn[:, 0 * C:1 * C][:, cs]).astype(bf16),
                "wk": np.ascontiguousarray(W_attn[:, 1 * C:2 * C][:, cs]).astype(bf16),
                "wv": np.ascontiguousarray(W_attn[:, 2 * C:3 * C][:, cs]).astype(bf16),
                "bq": np.ascontiguousarray(
                    b_attn[0 * C:1 * C][cs].reshape(HPC, HD).T
                ),
                "bk": np.ascontiguousarray(
                    b_attn[1 * C:2 * C][cs].reshape(HPC, HD).T
                ),
                "bv": np.ascontiguousarray(
                    np.broadcast_to(b_attn[2 * C:3 * C][cs], (P, HCOLS))
                ),
                "wp": np.ascontiguousarray(W_proj[cs, :]).astype(bf16),
                "negm": negm,
                "ones": np.ones((P, P), dtype=bf16),
            }
        )
    return in_maps


_NC_CACHE = {}


def get_nc(split_waits=True):
    key = ("nc", split_waits)
    if key not in _NC_CACHE:
        _NC_CACHE[key] = build_nc(split_waits)
    return _NC_CACHE[key]


def kernel(x, W_attn, b_attn, W_proj, b_proj):
    in_maps = make_core_inputs(x, W_attn, b_attn, W_proj, b_proj)
    nc = get_nc()
    res = run_bass_kernel_spmd(nc, in_maps, core_ids=list(range(8)))
    parts = [r["out"].astype(np.float32) for r in res.results]
    y = np.empty((B, T, C), dtype=np.float32)
    bpf = np.asarray(b_proj, dtype=np.float32)
    for b in range(B):
        y[b] = parts[4 * b] + parts[4 * b + 1] + parts[4 * b + 2] + parts[4 * b + 3]
        y[b] += bpf
    return y


if __name__ == "__main__":
    rng = np.random.default_rng(0)
    x = rng.standard_normal((B, T, C), dtype=np.float32)
    W_attn = rng.standard_normal((C, 3 * C), dtype=np.float32) / np.sqrt(C)
    b_attn = rng.standard_normal(3 * C).astype(np.float32) * 0.02
    W_proj = rng.standard_normal((C, C), dtype=np.float32) / np.sqrt(C)
    b_proj = rng.standard_normal(C).astype(np.float32) * 0.02
    y = kernel(x, W_attn, b_attn, W_proj, b_proj)
    print(y.shape, y.dtype, float(np.abs(y).mean()))


# revision 19
# speedup vs baseline: 1.0719x; 1.0719x over previous
"""Causal self-attention kernel for 8 Trainium2 NeuronCores — v2.

Problem: y = CausalSelfAttention(x) with B=2, T=2048, C=2048, 16 heads,
head_dim=128, fp32 in/out (bf16 internally; rel-err budget 2e-2).

Sharding (8 cores): core = (b, hg), b in {0,1} batch, hg in {0..3} head
group of 4 heads.  Each core computes QKV for its heads, attention, and a
partial c_proj; host sums the 4 partials per batch.

v2 structure (single fused pass, software-scheduled interleave):
  - x^T is streamed ONCE; Q,K,V projections computed per 512-col time
    block (tt) with h-major PSUM rotation (boot block tt=0 is ci-major so
    the PE ramps at DMA delivery speed).
  - attention for q-block qt runs interleaved with QKV of block tt=qt+1
    and c_proj of block qt-2, merged by a virtual-PE-time scheduler, so
    the scalar engine's exp work hides under PE matmuls.
  - attention processes heads in lockstep pairs with the AV matmul
    deferred one k-step, so the in-order PE queue never parks behind the
    exp it just issued.
  - softmax denominator: DVE accumulates sum of exp tiles (ea += e), one
    ones-matmul per (head, qt) reduces across partitions, fast-approx
    reciprocal, broadcast multiply.
Everything PE-facing is bf16 (1 elem/cycle like fp32r, but LDWEIGHTS and
DMA are 2x faster); PSUM accumulation fp32; output partials bf16.
"""

import numpy as np

import concourse.bass as bass
import concourse.mybir as mybir
import concourse.tile as tile
from concourse.bass_utils import run_bass_kernel_spmd

B, T, C = 2, 2048, 2048
N_HEAD = 16
HD = 128
HPC = 4          # heads per core
HCOLS = HPC * HD  # 512 columns of W per core per projection
P = 128          # partitions
QT = 512         # q-block (free dim) for projections and attention
KT = 128         # k-tile in attention
NQT = T // QT    # 4
NCT = C // P     # 16 contraction tiles for projections
SCALE = 1.0 / np.sqrt(HD)

F32 = mybir.dt.float32
F32R = mybir.dt.float32r
BF16 = mybir.dt.bfloat16
MMDT = BF16


def build_nc(split_waits=True):
    nc = bass.Bass("TRN2", target_bir_lowering=False, debug=False)

    xT = nc.dram_tensor("xT", [C, T], MMDT, kind="ExternalInput").ap()
    wq = nc.dram_tensor("wq", [C, HCOLS], MMDT, kind="ExternalInput").ap()
    wk = nc.dram_tensor("wk", [C, HCOLS], MMDT, kind="ExternalInput").ap()
    wv = nc.dram_tensor("wv", [C, HCOLS], MMDT, kind="ExternalInput").ap()
    bq = nc.dram_tensor("bq", [P, HPC], F32, kind="ExternalInput").ap()
    bk = nc.dram_tensor("bk", [P, HPC], F32, kind="ExternalInput").ap()
    bv = nc.dram_tensor("bv", [P, HCOLS], F32, kind="ExternalInput").ap()
    wp = nc.dram_tensor("wp", [HCOLS, C], MMDT, kind="ExternalInput").ap()
    # negm[r, c]: -1e30 where r > c-512 else 0.  The window
    # negm[:, 512-128j : 640-128j] is the additive causal mask for the
    # j-th diagonal k-tile of a 512-wide q-block (full -inf for the j
    # leading 128-col groups, lower-triangle on the diagonal group).
    negm = nc.dram_tensor("negm", [P, QT + P], F32, kind="ExternalInput").ap()
    ones = nc.dram_tensor("ones", [P, P], MMDT, kind="ExternalInput").ap()
    out = nc.dram_tensor("out", [T, C], MMDT, kind="ExternalOutput").ap()

    with tile.TileContext(nc) as tc:
        _build(tc, xT, wq, wk, wv, bq, bk, bv, wp, negm, ones, out)
    if split_waits:
        _split_matmul_waits(nc)
    return nc


def _split_matmul_waits(nc):
    """Lowered instructions fit only ONE sync-wait command (walrus: 'Too many
    sync wait commands').  Move excess waits onto preceding same-engine
    EventSemaphore instructions (which hold 2 waits each)."""
    n = 0
    for f in nc.m.functions:
        for b in f.blocks:
            patched = []
            changed = False
            for inst in b.instructions:
                si = inst.sync_info
                if (
                    not isinstance(inst, mybir.InstEventSemaphore)
                    and si is not None
                    and len(si.on_wait) > 1
                ):
                    waits = list(si.on_wait)
                    extra, keep = waits[:-1], waits[-1:]
                    for ci in range(0, len(extra), 2):
                        n += 1
                        patched.append(
                            mybir.InstEventSemaphore(
                                name=f"{inst.name}-wsplit{ci}",
                                engine=inst.engine,
                                ins=[],
                                outs=[],
                                sync_info=mybir.SyncInfo(
                                    on_wait=extra[ci:ci + 2], on_update=[]
                                ),
                            )
                        )
                    si.on_wait = keep
                    changed = True
                patched.append(inst)
            if changed:
                b.instructions = patched
    return n


def _merge(streams):
    """Emit thunks from several (cost, fn) lists, interleaved so each
    stream's cumulative-cost fraction advances evenly (virtual PE time)."""
    seqs = [s for s in streams if s]
    totals = [sum(c for c, _ in s) for s in seqs]
    pos = [0] * len(seqs)
    spent = [0.0] * len(seqs)
    while True:
        live = [i for i in range(len(seqs)) if pos[i] < len(seqs[i])]
        if not live:
            break
        i = min(live, key=lambda j: spent[j] / totals[j])
        c, fn = seqs[i][pos[i]]
        fn()
        spent[i] += c
        pos[i] += 1


def _build(tc, xT, wq, wk, wv, bq, bk, bv, wp, negm, ones, out):
    from contextlib import ExitStack

    nc = tc.nc
    Exp = mybir.ActivationFunctionType.Exp
    Ln = mybir.ActivationFunctionType.Ln
    Ident = mybir.ActivationFunctionType.Identity

    with ExitStack() as root:
        # ---- SBUF residents ----------------------------------------------
        res = root.enter_context(tc.tile_pool(name="res", bufs=1))
        qt_sb = res.tile([P, HPC, T], MMDT, tag="qt_sb")
        kt_sb = res.tile([P, HPC, T], MMDT, tag="kt_sb")
        v_sb = res.tile([P, NCT, HCOLS], MMDT, tag="v_sb")
        yt_sb = res.tile([P, HPC, T], MMDT, tag="yt_sb")
        wq_sb = res.tile([P, NCT, HCOLS], MMDT, tag="wq_sb")
        wk_sb = res.tile([P, NCT, HCOLS], MMDT, tag="wk_sb")
        wv_sb = res.tile([P, NCT, HCOLS], MMDT, tag="wv_sb")
        wp_sb = res.tile([P, HPC, C], MMDT, tag="wp_sb")
        bq_sb = res.tile([P, HPC], F32, tag="bq_sb")
        bk_sb = res.tile([P, HPC], F32, tag="bk_sb")
        bv_sb = res.tile([P, HCOLS], F32, tag="bv_sb")
        negm_sb = res.tile([P, QT + P], F32, tag="negm_sb")
        ones_sb = res.tile([P, P], MMDT, tag="ones_sb")

        xpool = root.enter_context(tc.tile_pool(name="xpool", bufs=2))
        epool = root.enter_context(tc.tile_pool(name="epool", bufs=4))
        eapool = root.enter_context(tc.tile_pool(name="eapool", bufs=2))
        rpool = root.enter_context(tc.tile_pool(name="rpool", bufs=2))
        opool = root.enter_context(tc.tile_pool(name="opool", bufs=2))

        # ---- input DMAs ---------------------------------------------------
        # sync queue: biases, then weight slabs in ci order (consumption
        # order of the ci-major boot block).  gpsimd queue: x tiles in ci
        # order, then masks/ones, then the tt=1 x prefetch.
        nc.sync.dma_start(out=bq_sb[:, :], in_=bq)
        nc.sync.dma_start(out=bk_sb[:, :], in_=bk)
        wqr = wq.rearrange("(co ci) n -> ci co n", ci=P)
        wkr = wk.rearrange("(co ci) n -> ci co n", ci=P)
        wvr = wv.rearrange("(co ci) n -> ci co n", ci=P)
        wpr = wp.rearrange("(ht p) c -> p ht c", p=P)

        xts = {}
        xTr = xT.rearrange("(ci p) t -> p ci t", p=P)

        def xt_load(tt, fine=False):
            t = xpool.tile([P, NCT, QT], MMDT, tag="xt", name=f"xt{tt}")
            for ci in range(NCT):
                nc.gpsimd.dma_start(
                    out=t[:, ci, :],
                    in_=xTr[:, ci, tt * QT:(tt + 1) * QT],
                )
            xts[tt] = t

        # boot block consumes (wq[ci], xt0[ci]) pairs at ~852ns each — split
        # both streams across the two DMA queues by parity so delivery keeps
        # pace with consumption (~150 GB/s per queue instead of 300 on one).
        t0 = xpool.tile([P, NCT, QT], MMDT, tag="xt", name="xt0")
        xts[0] = t0
        for ci in range(NCT):
            we, xe = (nc.sync, nc.gpsimd) if ci % 2 == 0 else (nc.gpsimd, nc.sync)
            we.dma_start(out=wq_sb[:, ci, :], in_=wqr[:, ci, :])
            xe.dma_start(out=t0[:, ci, :], in_=xTr[:, ci, 0:QT])
        for ci in range(NCT):
            eng = nc.sync if ci % 2 == 0 else nc.gpsimd
            eng.dma_start(out=wk_sb[:, ci, :], in_=wkr[:, ci, :])
        nc.sync.dma_start(out=bv_sb[:, :], in_=bv)
        for ci in range(NCT):
            eng = nc.sync if ci % 2 == 0 else nc.gpsimd
            eng.dma_start(out=wv_sb[:, ci, :], in_=wvr[:, ci, :])
        t1 = xpool.tile([P, NCT, QT], MMDT, tag="xt", name="xt1")
        xts[1] = t1
        for ci in range(NCT):
            eng = nc.sync if ci % 2 == 0 else nc.gpsimd
            eng.dma_start(out=t1[:, ci, :], in_=xTr[:, ci, QT:2 * QT])
        for i in range(HPC):
            nc.sync.dma_start(out=wp_sb[:, i, :], in_=wpr[:, i, :])
        nc.gpsimd.dma_start(out=negm_sb[:, :], in_=negm)
        nc.gpsimd.dma_start(out=ones_sb[:, :], in_=ones)

        # ---- boot block: QKV for tt=0, ci-major (consume DMA in order) ---
        with ExitStack() as bootctx:
            boot_psum = bootctx.enter_context(
                tc.tile_pool(name="boot_psum", bufs=6, space="PSUM")
            )
            xt0 = xts[0]
            for kind, w_sb, drain in (
                ("q", wq_sb, "q"), ("k", wk_sb, "k"), ("v", wv_sb, "v"),
            ):
                ps = [
                    boot_psum.tile([P, QT], F32, tag="boot", name=f"b{kind}{i}")
                    for i in range(4)
                ]
                for ci in range(NCT):
                    for i in range(4):
                        if kind == "v":
                            lhsT = xt0[:, ci, i * P:(i + 1) * P]
                            rhs = wv_sb[:, ci, :]
                        else:
                            lhsT = w_sb[:, ci, i * HD:(i + 1) * HD]
                            rhs = xt0[:, ci, :]
                        nc.tensor.matmul(
                            ps[i][:, :], lhsT, rhs,
                            start=(ci == 0), stop=(ci == NCT - 1),
                        )
                for i in range(4):
                    if kind == "q":
                        nc.scalar.activation(
                            qt_sb[:, i, 0:QT], ps[i][:, :], Ident,
                            bias=bq_sb[:, i:i + 1],
                        )
                    elif kind == "k":
                        nc.scalar.activation(
                            kt_sb[:, i, 0:QT], ps[i][:, :], Ident,
                            bias=bk_sb[:, i:i + 1],
                        )
                    else:
                        nc.vector.tensor_add(
                            v_sb[:, i, :], ps[i][:, :], bv_sb[:, :]
                        )

        # ---- main pools ---------------------------------------------------
        pq_psum = root.enter_context(tc.tile_pool(name="pq_psum", bufs=2, space="PSUM"))
        s_psum = root.enter_context(tc.tile_pool(name="s_psum", bufs=2, space="PSUM"))
        y_psum = root.enter_context(tc.tile_pool(name="y_psum", bufs=2, space="PSUM"))
        d_psum = root.enter_context(tc.tile_pool(name="d_psum", bufs=2, space="PSUM"))

        # ---- stream builders ---------------------------------------------
        def qkv_stream(tt):
            """QKV projections for block tt, h-major (weights resident)."""
            th = []
            if tt + 1 < NQT:
                th.append((0.0, lambda tt=tt: xt_load(tt + 1)))
            state = {}

            def quad(kind, i, cq, tt=tt):
                def fn(kind=kind, i=i, cq=cq, tt=tt):
                    if cq == 0:
                        state[(kind, i)] = pq_psum.tile(
                            [P, QT], F32, tag="pq", name=f"p{kind}{tt}_{i}"
                        )
                    ps = state[(kind, i)]
                    xt_t = xts[tt]
                    for ci in range(cq * 4, cq * 4 + 4):
                        if kind == "v":
                            lhsT = xt_t[:, ci, i * P:(i + 1) * P]
                            rhs = wv_sb[:, ci, :]
                        elif kind == "q":
                            lhsT = wq_sb[:, ci, i * HD:(i + 1) * HD]
                            rhs = xt_t[:, ci, :]
                        else:
                            lhsT = wk_sb[:, ci, i * HD:(i + 1) * HD]
                            rhs = xt_t[:, ci, :]
                        nc.tensor.matmul(
                            ps[:, :], lhsT, rhs,
                            start=(ci == 0), stop=(ci == NCT - 1),
                        )
                    if cq == 3:
                        if kind == "q":
                            nc.scalar.activation(
                                qt_sb[:, i, tt * QT:(tt + 1) * QT], ps[:, :],
                                Ident, bias=bq_sb[:, i:i + 1],
                            )
                        elif kind == "k":
                            nc.scalar.activation(
                                kt_sb[:, i, tt * QT:(tt + 1) * QT], ps[:, :],
                                Ident, bias=bk_sb[:, i:i + 1],
                            )
                        else:
                            nc.vector.tensor_add(
                                v_sb[:, tt * 4 + i, :], ps[:, :], bv_sb[:, :]
                            )
                return fn

            for kind in ("q", "k", "v"):
                for i in range(4):
                    for cq in range(4):
                        th.append((852.0, quad(kind, i, cq)))
            return th

        def attn_stream(qt):
            """Attention for q-block qt: head pairs in lockstep, AV matmul
            deferred one k-step behind the S matmul + exp.  Causal mask is
            added to S in PSUM (windowed -1e30 const) BEFORE exp, so the
            exp output feeds the AV matmul with no DVE op in between."""
            nkt = 4 * qt + 4
            th = []
            for hp in range(2):
                h0 = 2 * hp
                st = {}

                def stepA(h, kt, qt=qt, st=st):
                    def fn(h=h, kt=kt, qt=qt, st=st):
                        if kt == 0:
                            st[("y", h)] = y_psum.tile(
                                [P, QT], F32, tag="y", name=f"y{h}_{qt}"
                            )
                            st[("ea", h)] = eapool.tile(
                                [P, QT], MMDT, tag="ea", name=f"ea{h}_{qt}"
                            )
                        # diagonal k-tiles: columns [0, j*128) are fully
                        # masked — compute only the live window [w0, QT)
                        j = kt - 4 * qt
                        w0 = max(j, 0) * P
                        s_ps = s_psum.tile([P, QT], F32, tag="s", name=f"s{h}_{kt}")
                        nc.tensor.matmul(
                            s_ps[:, w0:QT],
                            kt_sb[:, h, kt * KT:(kt + 1) * KT],
                            qt_sb[:, h, qt * QT + w0:(qt + 1) * QT],
                            start=True, stop=True,
                        )
                        if j >= 0:
                            nc.vector.tensor_add(
                                s_ps[:, w0:w0 + P], s_ps[:, w0:w0 + P],
                                negm_sb[:, QT:QT + P],
                            )
                        e_sb = epool.tile([P, QT], MMDT, tag="e", name=f"e{h}_{kt}")
                        nc.scalar.activation(
                            e_sb[:, w0:QT], s_ps[:, w0:QT], Exp, scale=float(SCALE)
                        )
                        st[("e", h, kt)] = (e_sb, w0)
                    return fn

                def stepB(h, kt, qt=qt, st=st, nkt=nkt):
                    def fn(h=h, kt=kt, qt=qt, st=st, nkt=nkt):
                        e_sb, w0 = st.pop(("e", h, kt))
                        ea_sb = st[("ea", h)]
                        if kt == 0:
                            nc.vector.tensor_copy(ea_sb[:, :], e_sb[:, :])
                        else:
                            nc.vector.tensor_add(
                                ea_sb[:, w0:QT], ea_sb[:, w0:QT], e_sb[:, w0:QT]
                            )
                        y_ps = st[("y", h)]
                        nc.tensor.matmul(
                            y_ps[:, w0:QT],
                            v_sb[:, kt, h * HD:(h + 1) * HD],
                            e_sb[:, w0:QT],
                            start=(kt == 0), stop=(kt == nkt - 1),
                            skip_group_check=(w0 > 0),
                        )
                        if kt == nkt - 1:
                            den_ps = d_psum.tile(
                                [P, QT], F32, tag="den", name=f"den{h}_{qt}"
                            )
                            nc.tensor.matmul(
                                den_ps[:, :], ones_sb[:, :], ea_sb[:, :],
                                start=True, stop=True,
                            )
                            # 1/den = exp(-ln(den)) on ACT (ln and exp share
                            # the natural_log_exp_and_others table set) —
                            # ~5x cheaper than DVE's iterative reciprocal
                            rln = rpool.tile(
                                [P, QT], F32, tag="rln", name=f"rln{h}_{qt}"
                            )
                            nc.scalar.activation(rln[:, :], den_ps[:, :], Ln)
                            rbc = rpool.tile(
                                [P, QT], F32, tag="rbc", name=f"rbc{h}_{qt}"
                            )
                            nc.scalar.activation(
                                rbc[:, :], rln[:, :], Exp, scale=-1.0
                            )
                            nc.vector.tensor_mul(
                                yt_sb[:, h, qt * QT:(qt + 1) * QT],
                                y_ps[:, :], rbc[:, :],
                            )
                    return fn

                # lockstep pair, B deferred one k-step
                for kt in range(nkt):
                    th.append((213.0, stepA(h0, kt)))
                    th.append((213.0, stepA(h0 + 1, kt)))
                    if kt > 0:
                        th.append((213.0, stepB(h0, kt - 1)))
                        th.append((213.0, stepB(h0 + 1, kt - 1)))
                th.append((426.0, stepB(h0, nkt - 1)))
                th.append((426.0, stepB(h0 + 1, nkt - 1)))
            return th

        def proj_stream(qt):
            """Partial c_proj for the four 128-row tiles of q-block qt."""
            th = []
            state = {}

            def tile_fn(qi, ct):
                def fn(qi=qi, ct=ct):
                    if ct == 0:
                        state[qi] = opool.tile([P, C], MMDT, tag="o", name=f"o{qi}")
                    o_sb = state[qi]
                    cp = pq_psum.tile([P, QT], F32, tag="pq", name=f"cp{qi}_{ct}")
                    for h in range(HPC):
                        nc.tensor.matmul(
                            cp[:, :],
                            yt_sb[:, h, qi * P:(qi + 1) * P],
                            wp_sb[:, h, ct * QT:(ct + 1) * QT],
                            start=(h == 0), stop=(h == HPC - 1),
                        )
                    nc.vector.tensor_copy(o_sb[:, ct * QT:(ct + 1) * QT], cp[:, :])
                    if ct == 3:
                        nc.sync.dma_start(
                            out=out[qi * P:(qi + 1) * P, :], in_=o_sb[:, :]
                        )
                return fn

            for qi in range(4 * qt, 4 * qt + 4):
                for ct in range(4):
                    th.append((852.0, tile_fn(qi, ct)))
            return th

        # ---- schedule -----------------------------------------------------
        _merge([qkv_stream(1), attn_stream(0)])
        _merge([qkv_stream(2), attn_stream(1), proj_stream(0)])
        _merge([qkv_stream(3), attn_stream(2), proj_stream(1)])
        _merge([attn_stream(3), proj_stream(2)])
        _merge([proj_stream(3)])


def make_core_inputs(x, W_attn, b_attn, W_proj, b_proj):
    """Host-side shard/prep. Returns list of 8 input dicts."""
    import ml_dtypes

    bf16 = ml_dtypes.bfloat16
    x = np.asarray(x, dtype=np.float32)
    W_attn = np.asarray(W_attn, dtype=np.float32)
    b_attn = np.asarray(b_attn, dtype=np.float32)
    W_proj = np.asarray(W_proj, dtype=np.float32)
    b_proj = np.asarray(b_proj, dtype=np.float32)

    r = np.arange(P)[:, None]
    c = np.arange(QT + P)[None, :]
    negm = np.where(r > c - QT, np.float32(-1e30), np.float32(0.0))
    negm = np.ascontiguousarray(negm.astype(np.float32))

    in_maps = []
    for core in range(8):
        b, hg = divmod(core, 4)
        cs = slice(HCOLS * hg, HCOLS * hg + HCOLS)
        in_maps.append(
            {
                "xT": np.ascontiguousarray(x[b].T).astype(bf16),
                "wq": np.ascontiguousarray(W_attn[:, 0 * C:1 * C][:, cs]).astype(bf16),
                "wk": np.ascontiguousarray(W_attn[:, 1 * C:2 * C][:, cs]).astype(bf16),
                "wv": np.ascontiguousarray(W_attn[:, 2 * C:3 * C][:, cs]).astype(bf16),
                "bq": np.ascontiguousarray(
                    b_attn[0 * C:1 * C][cs].reshape(HPC, HD).T
                ),
                "bk": np.ascontiguousarray(
                    b_attn[1 * C:2 * C][cs].reshape(HPC, HD).T
                ),
                "bv": np.ascontiguousarray(
                    np.broadcast_to(b_attn[2 * C:3 * C][cs], (P, HCOLS))
                ),
                "wp": np.ascontiguousarray(W_proj[cs, :]).astype(bf16),
                "negm": negm,
                "ones": np.ones((P, P), dtype=bf16),
            }
        )
    return in_maps


_NC_CACHE = {}


def get_nc(split_waits=True):
    key = ("nc", split_waits)
    if key not in _NC_CACHE:
        _NC_CACHE[key] = build_nc(split_waits)
    return _NC_CACHE[key]


def kernel(x, W_attn, b_attn, W_proj, b_proj):
    in_maps = make_core_inputs(x, W_attn, b_attn, W_proj, b_proj)
    nc = get_nc()
    res = run_bass_kernel_spmd(nc, in_maps, core_ids=list(range(8)))
    parts = [r["out"].astype(np.float32) for r in res.results]
    y = np.empty((B, T, C), dtype=np.float32)
    bpf = np.asarray(b_proj, dtype=np.float32)
    for b in range(B):
        y[b] = parts[4 * b] + parts[4 * b + 1] + parts[4 * b + 2] + parts[4 * b + 3]
        y[b] += bpf
    return y


if __name__ == "__main__":
    rng = np.random.default_rng(0)
    x = rng.standard_normal((B, T, C), dtype=np.float32)
    W_attn = rng.standard_normal((C, 3 * C), dtype=np.float32) / np.sqrt(C)
    b_attn = rng.standard_normal(3 * C).astype(np.float32) * 0.02
    W_proj = rng.standard_normal((C, C), dtype=np.float32) / np.sqrt(C)
    b_proj = rng.standard_normal(C).astype(np.float32) * 0.02
    y = kernel(x, W_attn, b_attn, W_proj, b_proj)
    print(y.shape, y.dtype, float(np.abs(y).mean()))


# revision 23
# speedup vs baseline: 1.0887x; 1.0157x over previous
"""Causal self-attention kernel for 8 Trainium2 NeuronCores — v2.

Problem: y = CausalSelfAttention(x) with B=2, T=2048, C=2048, 16 heads,
head_dim=128, fp32 in/out (bf16 internally; rel-err budget 2e-2).

Sharding (8 cores): core = (b, hg), b in {0,1} batch, hg in {0..3} head
group of 4 heads.  Each core computes QKV for its heads, attention, and a
partial c_proj; host sums the 4 partials per batch.

v2 structure (single fused pass, software-scheduled interleave):
  - x^T is streamed ONCE; Q,K,V projections computed per 512-col time
    block (tt) with h-major PSUM rotation (boot block tt=0 is ci-major so
    the PE ramps at DMA delivery speed).
  - attention for q-block qt runs interleaved with QKV of block tt=qt+1
    and c_proj of block qt-2, merged by a virtual-PE-time scheduler, so
    the scalar engine's exp work hides under PE matmuls.
  - attention processes heads in lockstep pairs with the AV matmul
    deferred one k-step, so the in-order PE queue never parks behind the
    exp it just issued.
  - softmax denominator: DVE accumulates sum of exp tiles (ea += e), one
    ones-matmul per (head, qt) reduces across partitions, fast-approx
    reciprocal, broadcast multiply.
Everything PE-facing is bf16 (1 elem/cycle like fp32r, but LDWEIGHTS and
DMA are 2x faster); PSUM accumulation fp32; output partials bf16.
"""

import numpy as np

import concourse.bass as bass
import concourse.mybir as mybir
import concourse.tile as tile
from concourse.bass_utils import run_bass_kernel_spmd

B, T, C = 2, 2048, 2048
N_HEAD = 16
HD = 128
HPC = 4          # heads per core
HCOLS = HPC * HD  # 512 columns of W per core per projection
P = 128          # partitions
QT = 512         # q-block (free dim) for projections and attention
KT = 128         # k-tile in attention
NQT = T // QT    # 4
NCT = C // P     # 16 contraction tiles for projections
SCALE = 1.0 / np.sqrt(HD)

F32 = mybir.dt.float32
F32R = mybir.dt.float32r
BF16 = mybir.dt.bfloat16
MMDT = BF16


def build_nc(split_waits=True):
    nc = bass.Bass("TRN2", target_bir_lowering=False, debug=False)

    xT = nc.dram_tensor("xT", [C, T], MMDT, kind="ExternalInput").ap()
    wq = nc.dram_tensor("wq", [C, HCOLS], MMDT, kind="ExternalInput").ap()
    wk = nc.dram_tensor("wk", [C, HCOLS], MMDT, kind="ExternalInput").ap()
    wv = nc.dram_tensor("wv", [C, HCOLS], MMDT, kind="ExternalInput").ap()
    bq = nc.dram_tensor("bq", [P, HPC], F32, kind="ExternalInput").ap()
    bk = nc.dram_tensor("bk", [P, HPC], F32, kind="ExternalInput").ap()
    bv = nc.dram_tensor("bv", [P, HCOLS], F32, kind="ExternalInput").ap()
    wp = nc.dram_tensor("wp", [HCOLS, C], MMDT, kind="ExternalInput").ap()
    # negm[r, c]: -1e30 where r > c-512 else 0.  The window
    # negm[:, 512-128j : 640-128j] is the additive causal mask for the
    # j-th diagonal k-tile of a 512-wide q-block (full -inf for the j
    # leading 128-col groups, lower-triangle on the diagonal group).
    negm = nc.dram_tensor("negm", [P, QT + P], F32, kind="ExternalInput").ap()
    ones = nc.dram_tensor("ones", [P, P], MMDT, kind="ExternalInput").ap()
    out = nc.dram_tensor("out", [T, C], MMDT, kind="ExternalOutput").ap()

    with tile.TileContext(nc) as tc:
        _build(tc, xT, wq, wk, wv, bq, bk, bv, wp, negm, ones, out)
    if split_waits:
        _split_matmul_waits(nc)
    return nc


def _split_matmul_waits(nc):
    """Lowered instructions fit only ONE sync-wait command (walrus: 'Too many
    sync wait commands').  Move excess waits onto preceding same-engine
    EventSemaphore instructions (which hold 2 waits each)."""
    n = 0
    for f in nc.m.functions:
        for b in f.blocks:
            patched = []
            changed = False
            for inst in b.instructions:
                si = inst.sync_info
                if (
                    not isinstance(inst, mybir.InstEventSemaphore)
                    and si is not None
                    and len(si.on_wait) > 1
                ):
                    waits = list(si.on_wait)
                    extra, keep = waits[:-1], waits[-1:]
                    for ci in range(0, len(extra), 2):
                        n += 1
                        patched.append(
                            mybir.InstEventSemaphore(
                                name=f"{inst.name}-wsplit{ci}",
                                engine=inst.engine,
                                ins=[],
                                outs=[],
                                sync_info=mybir.SyncInfo(
                                    on_wait=extra[ci:ci + 2], on_update=[]
                                ),
                            )
                        )
                    si.on_wait = keep
                    changed = True
                patched.append(inst)
            if changed:
                b.instructions = patched
    return n


def _merge(streams):
    """Emit thunks from several (cost, fn) lists, interleaved so each
    stream's cumulative-cost fraction advances evenly (virtual PE time)."""
    seqs = [s for s in streams if s]
    totals = [sum(c for c, _ in s) for s in seqs]
    pos = [0] * len(seqs)
    spent = [0.0] * len(seqs)
    while True:
        live = [i for i in range(len(seqs)) if pos[i] < len(seqs[i])]
        if not live:
            break
        i = min(live, key=lambda j: spent[j] / totals[j])
        c, fn = seqs[i][pos[i]]
        fn()
        spent[i] += c
        pos[i] += 1


def _build(tc, xT, wq, wk, wv, bq, bk, bv, wp, negm, ones, out):
    from contextlib import ExitStack

    nc = tc.nc
    Exp = mybir.ActivationFunctionType.Exp
    Ln = mybir.ActivationFunctionType.Ln
    Ident = mybir.ActivationFunctionType.Identity

    with ExitStack() as root:
        # ---- SBUF residents ----------------------------------------------
        res = root.enter_context(tc.tile_pool(name="res", bufs=1))
        qt_sb = res.tile([P, HPC, T], MMDT, tag="qt_sb")
        kt_sb = res.tile([P, HPC, T], MMDT, tag="kt_sb")
        v_sb = res.tile([P, NCT, HCOLS], MMDT, tag="v_sb")
        yt_sb = res.tile([P, HPC, T], MMDT, tag="yt_sb")
        wq_sb = res.tile([P, NCT, HCOLS], MMDT, tag="wq_sb")
        wk_sb = res.tile([P, NCT, HCOLS], MMDT, tag="wk_sb")
        wv_sb = res.tile([P, NCT, HCOLS], MMDT, tag="wv_sb")
        wp_sb = res.tile([P, HPC, C], MMDT, tag="wp_sb")
        bq_sb = res.tile([P, HPC], F32, tag="bq_sb")
        bk_sb = res.tile([P, HPC], F32, tag="bk_sb")
        bv_sb = res.tile([P, HCOLS], F32, tag="bv_sb")
        negm_sb = res.tile([P, QT + P], F32, tag="negm_sb")
        ones_sb = res.tile([P, P], MMDT, tag="ones_sb")

        xpool = root.enter_context(tc.tile_pool(name="xpool", bufs=2))
        epool = root.enter_context(tc.tile_pool(name="epool", bufs=5))
        eapool = root.enter_context(tc.tile_pool(name="eapool", bufs=2))
        rpool = root.enter_context(tc.tile_pool(name="rpool", bufs=2))
        opool = root.enter_context(tc.tile_pool(name="opool", bufs=2))

        # ---- input DMAs ---------------------------------------------------
        # sync queue: biases, then weight slabs in ci order (consumption
        # order of the ci-major boot block).  gpsimd queue: x tiles in ci
        # order, then masks/ones, then the tt=1 x prefetch.
        nc.sync.dma_start(out=bq_sb[:, :], in_=bq)
        nc.sync.dma_start(out=bk_sb[:, :], in_=bk)
        wqr = wq.rearrange("(co ci) n -> ci co n", ci=P)
        wkr = wk.rearrange("(co ci) n -> ci co n", ci=P)
        wvr = wv.rearrange("(co ci) n -> ci co n", ci=P)
        wpr = wp.rearrange("(ht p) c -> p ht c", p=P)

        xts = {}
        xTr = xT.rearrange("(ci p) t -> p ci t", p=P)

        def xt_load(tt, fine=False):
            t = xpool.tile([P, NCT, QT], MMDT, tag="xt", name=f"xt{tt}")
            for ci in range(NCT):
                nc.gpsimd.dma_start(
                    out=t[:, ci, :],
                    in_=xTr[:, ci, tt * QT:(tt + 1) * QT],
                )
            xts[tt] = t

        # boot block consumes (wq[ci], xt0[ci]) pairs at ~852ns each — split
        # both streams across the two DMA queues by parity so delivery keeps
        # pace with consumption (~150 GB/s per queue instead of 300 on one).
        t0 = xpool.tile([P, NCT, QT], MMDT, tag="xt", name="xt0")
        xts[0] = t0
        for ci in range(NCT):
            we, xe = (nc.sync, nc.gpsimd) if ci % 2 == 0 else (nc.gpsimd, nc.sync)
            we.dma_start(out=wq_sb[:, ci, :], in_=wqr[:, ci, :])
            xe.dma_start(out=t0[:, ci, :], in_=xTr[:, ci, 0:QT])
        for ci in range(NCT):
            eng = nc.sync if ci % 2 == 0 else nc.gpsimd
            eng.dma_start(out=wk_sb[:, ci, :], in_=wkr[:, ci, :])
        nc.sync.dma_start(out=bv_sb[:, :], in_=bv)
        for ci in range(NCT):
            eng = nc.sync if ci % 2 == 0 else nc.gpsimd
            eng.dma_start(out=wv_sb[:, ci, :], in_=wvr[:, ci, :])
        t1 = xpool.tile([P, NCT, QT], MMDT, tag="xt", name="xt1")
        xts[1] = t1
        for ci in range(NCT):
            eng = nc.sync if ci % 2 == 0 else nc.gpsimd
            eng.dma_start(out=t1[:, ci, :], in_=xTr[:, ci, QT:2 * QT])
        for i in range(HPC):
            nc.sync.dma_start(out=wp_sb[:, i, :], in_=wpr[:, i, :])
        nc.gpsimd.dma_start(out=negm_sb[:, :], in_=negm)
        nc.gpsimd.dma_start(out=ones_sb[:, :], in_=ones)

        # ---- boot block: QKV for tt=0, ci-major (consume DMA in order) ---
        with ExitStack() as bootctx:
            boot_psum = bootctx.enter_context(
                tc.tile_pool(name="boot_psum", bufs=6, space="PSUM")
            )
            xt0 = xts[0]
            for kind, w_sb, drain in (
                ("q", wq_sb, "q"), ("k", wk_sb, "k"), ("v", wv_sb, "v"),
            ):
                ps = [
                    boot_psum.tile([P, QT], F32, tag="boot", name=f"b{kind}{i}")
                    for i in range(4)
                ]
                for ci in range(NCT):
                    for i in range(4):
                        if kind == "v":
                            lhsT = xt0[:, ci, i * P:(i + 1) * P]
                            rhs = wv_sb[:, ci, :]
                        else:
                            lhsT = w_sb[:, ci, i * HD:(i + 1) * HD]
                            rhs = xt0[:, ci, :]
                        nc.tensor.matmul(
                            ps[i][:, :], lhsT, rhs,
                            start=(ci == 0), stop=(ci == NCT - 1),
                        )
                for i in range(4):
                    if kind == "q":
                        nc.scalar.activation(
                            qt_sb[:, i, 0:QT], ps[i][:, :], Ident,
                            bias=bq_sb[:, i:i + 1],
                        )
                    elif kind == "k":
                        nc.scalar.activation(
                            kt_sb[:, i, 0:QT], ps[i][:, :], Ident,
                            bias=bk_sb[:, i:i + 1],
                        )
                    else:
                        nc.vector.tensor_add(
                            v_sb[:, i, :], ps[i][:, :], bv_sb[:, :]
                        )

        # ---- main pools ---------------------------------------------------
        pq_psum = root.enter_context(tc.tile_pool(name="pq_psum", bufs=2, space="PSUM"))
        s_psum = root.enter_context(tc.tile_pool(name="s_psum", bufs=2, space="PSUM"))
        y_psum = root.enter_context(tc.tile_pool(name="y_psum", bufs=2, space="PSUM"))
        d_psum = root.enter_context(tc.tile_pool(name="d_psum", bufs=2, space="PSUM"))

        # ---- stream builders ---------------------------------------------
        def qkv_stream(tt):
            """QKV projections for block tt, h-major (weights resident)."""
            th = []
            if tt + 1 < NQT:
                th.append((0.0, lambda tt=tt: xt_load(tt + 1)))
            state = {}

            def quad(kind, i, cq, tt=tt):
                def fn(kind=kind, i=i, cq=cq, tt=tt):
                    if cq == 0:
                        state[(kind, i)] = pq_psum.tile(
                            [P, QT], F32, tag="pq", name=f"p{kind}{tt}_{i}"
                        )
                    ps = state[(kind, i)]
                    xt_t = xts[tt]
                    for ci in range(cq * 4, cq * 4 + 4):
                        if kind == "v":
                            lhsT = xt_t[:, ci, i * P:(i + 1) * P]
                            rhs = wv_sb[:, ci, :]
                        elif kind == "q":
                            lhsT = wq_sb[:, ci, i * HD:(i + 1) * HD]
                            rhs = xt_t[:, ci, :]
                        else:
                            lhsT = wk_sb[:, ci, i * HD:(i + 1) * HD]
                            rhs = xt_t[:, ci, :]
                        nc.tensor.matmul(
                            ps[:, :], lhsT, rhs,
                            start=(ci == 0), stop=(ci == NCT - 1),
                        )
                    if cq == 3:
                        # drains on DVE, keeping the ACT queue free for exps
                        # (a drain stuck behind exps stalls the pq PSUM ring)
                        if kind == "q":
                            nc.vector.tensor_scalar_add(
                                qt_sb[:, i, tt * QT:(tt + 1) * QT], ps[:, :],
                                bq_sb[:, i:i + 1],
                            )
                        elif kind == "k":
                            nc.vector.tensor_scalar_add(
                                kt_sb[:, i, tt * QT:(tt + 1) * QT], ps[:, :],
                                bk_sb[:, i:i + 1],
                            )
                        else:
                            nc.vector.tensor_add(
                                v_sb[:, tt * 4 + i, :], ps[:, :], bv_sb[:, :]
                            )
                return fn

            for kind in ("q", "k", "v"):
                for i in range(4):
                    for cq in range(4):
                        th.append((852.0, quad(kind, i, cq)))
            return th

        def attn_stream(qt):
            """Attention for q-block qt: head pairs in lockstep, AV matmul
            deferred one k-step behind the S matmul + exp.  Causal mask is
            added to S in PSUM (windowed -1e30 const) BEFORE exp, so the
            exp output feeds the AV matmul with no DVE op in between."""
            nkt = 4 * qt + 4
            th = []
            for hp in range(2):
                h0 = 2 * hp
                st = {}

                def stepA(h, kt, qt=qt, st=st):
                    def fn(h=h, kt=kt, qt=qt, st=st):
                        if kt == 0:
                            st[("y", h)] = y_psum.tile(
                                [P, QT], F32, tag="y", name=f"y{h}_{qt}"
                            )
                            st[("ea", h)] = eapool.tile(
                                [P, QT], MMDT, tag="ea", name=f"ea{h}_{qt}"
                            )
                        # diagonal k-tiles: columns [0, j*128) are fully
                        # masked — compute only the live window [w0, QT)
                        j = kt - 4 * qt
                        w0 = max(j, 0) * P
                        s_ps = s_psum.tile([P, QT], F32, tag="s", name=f"s{h}_{kt}")
                        nc.tensor.matmul(
                            s_ps[:, w0:QT],
                            kt_sb[:, h, kt * KT:(kt + 1) * KT],
                            qt_sb[:, h, qt * QT + w0:(qt + 1) * QT],
                            start=True, stop=True,
                        )
                        if j >= 0:
                            nc.vector.tensor_add(
                                s_ps[:, w0:w0 + P], s_ps[:, w0:w0 + P],
                                negm_sb[:, QT:QT + P],
                            )
                        e_sb = epool.tile([P, QT], MMDT, tag="e", name=f"e{h}_{kt}")
                        nc.scalar.activation(
                            e_sb[:, w0:QT], s_ps[:, w0:QT], Exp, scale=float(SCALE)
                        )
                        st[("e", h, kt)] = (e_sb, w0)
                    return fn

                def stepB(h, kt, qt=qt, st=st, nkt=nkt):
                    def fn(h=h, kt=kt, qt=qt, st=st, nkt=nkt):
                        e_sb, w0 = st.pop(("e", h, kt))
                        ea_sb = st[("ea", h)]
                        if kt == 0:
                            nc.vector.tensor_copy(ea_sb[:, :], e_sb[:, :])
                        else:
                            nc.vector.tensor_add(
                                ea_sb[:, w0:QT], ea_sb[:, w0:QT], e_sb[:, w0:QT]
                            )
                        y_ps = st[("y", h)]
                        nc.tensor.matmul(
                            y_ps[:, w0:QT],
                            v_sb[:, kt, h * HD:(h + 1) * HD],
                            e_sb[:, w0:QT],
                            start=(kt == 0), stop=(kt == nkt - 1),
                            skip_group_check=(w0 > 0),
                        )
                        if kt == nkt - 1:
                            den_ps = d_psum.tile(
                                [P, QT], F32, tag="den", name=f"den{h}_{qt}"
                            )
                            nc.tensor.matmul(
                                den_ps[:, :], ones_sb[:, :], ea_sb[:, :],
                                start=True, stop=True,
                            )
                            # 1/den = exp(-ln(den)) on ACT (ln and exp share
                            # the natural_log_exp_and_others table set) —
                            # ~5x cheaper than DVE's iterative reciprocal
                            rln = rpool.tile(
                                [P, QT], F32, tag="rln", name=f"rln{h}_{qt}"
                            )
                            nc.scalar.activation(rln[:, :], den_ps[:, :], Ln)
                            rbc = rpool.tile(
                                [P, QT], F32, tag="rbc", name=f"rbc{h}_{qt}"
                            )
                            nc.scalar.activation(
                                rbc[:, :], rln[:, :], Exp, scale=-1.0
                            )
                            nc.vector.tensor_mul(
                                yt_sb[:, h, qt * QT:(qt + 1) * QT],
                                y_ps[:, :], rbc[:, :],
                            )
                    return fn

                # lockstep pair, B deferred two k-steps so the in-order PE
                # queue never reaches an AV matmul before its exp finished
                for kt in range(nkt):
                    th.append((213.0, stepA(h0, kt)))
                    th.append((213.0, stepA(h0 + 1, kt)))
                    if kt > 1:
                        th.append((213.0, stepB(h0, kt - 2)))
                        th.append((213.0, stepB(h0 + 1, kt - 2)))
                for kt in (nkt - 2, nkt - 1):
                    th.append((426.0, stepB(h0, kt)))
                    th.append((426.0, stepB(h0 + 1, kt)))
            return th

        def proj_stream(qt):
            """Partial c_proj for the four 128-row tiles of q-block qt."""
            th = []
            state = {}

            def tile_fn(qi, ct):
                def fn(qi=qi, ct=ct):
                    if ct == 0:
                        state[qi] = opool.tile([P, C], MMDT, tag="o", name=f"o{qi}")
                    o_sb = state[qi]
                    cp = pq_psum.tile([P, QT], F32, tag="pq", name=f"cp{qi}_{ct}")
                    for h in range(HPC):
                        nc.tensor.matmul(
                            cp[:, :],
                            yt_sb[:, h, qi * P:(qi + 1) * P],
                            wp_sb[:, h, ct * QT:(ct + 1) * QT],
                            start=(h == 0), stop=(h == HPC - 1),
                        )
                    nc.vector.tensor_copy(o_sb[:, ct * QT:(ct + 1) * QT], cp[:, :])
                    if ct == 3:
                        nc.sync.dma_start(
                            out=out[qi * P:(qi + 1) * P, :], in_=o_sb[:, :]
                        )
                return fn

            for qi in range(4 * qt, 4 * qt + 4):
                for ct in range(4):
                    th.append((852.0, tile_fn(qi, ct)))
            return th

        # ---- schedule -----------------------------------------------------
        # proj(1)+proj(2) ride in the attn(3) segment: that segment is
        # otherwise ACT-bound (64 exps), so it needs the extra PE work.
        _merge([qkv_stream(1), attn_stream(0)])
        _merge([qkv_stream(2), attn_stream(1), proj_stream(0)])
        _merge([qkv_stream(3), attn_stream(2)])
        _merge([attn_stream(3), proj_stream(1), proj_stream(2)])
        _merge([proj_stream(3)])


def make_core_inputs(x, W_attn, b_attn, W_proj, b_proj):
    """Host-side shard/prep. Returns list of 8 input dicts."""
    import ml_dtypes

    bf16 = ml_dtypes.bfloat16
    x = np.asarray(x, dtype=np.float32)
    W_attn = np.asarray(W_attn, dtype=np.float32)
    b_attn = np.asarray(b_attn, dtype=np.float32)
    W_proj = np.asarray(W_proj, dtype=np.float32)
    b_proj = np.asarray(b_proj, dtype=np.float32)

    r = np.arange(P)[:, None]
    c = np.arange(QT + P)[None, :]
    negm = np.where(r > c - QT, np.float32(-1e30), np.float32(0.0))
    negm = np.ascontiguousarray(negm.astype(np.float32))

    in_maps = []
    for core in range(8):
        b, hg = divmod(core, 4)
        cs = slice(HCOLS * hg, HCOLS * hg + HCOLS)
        in_maps.append(
            {
                "xT": np.ascontiguousarray(x[b].T).astype(bf16),
                "wq": np.ascontiguousarray(W_attn[:, 0 * C:1 * C][:, cs]).astype(bf16),
                "wk": np.ascontiguousarray(W_attn[:, 1 * C:2 * C][:, cs]).astype(bf16),
                "wv": np.ascontiguousarray(W_attn[:, 2 * C:3 * C][:, cs]).astype(bf16),
                "bq": np.ascontiguousarray(
                    b_attn[0 * C:1 * C][cs].reshape(HPC, HD).T
                ),
                "bk": np.ascontiguousarray(
                    b_attn[1 * C:2 * C][cs].reshape(HPC, HD).T
                ),
                "bv": np.ascontiguousarray(
                    np.broadcast_to(b_attn[2 * C:3 * C][cs], (P, HCOLS))
                ),
                "wp": np.ascontiguousarray(W_proj[cs, :]).astype(bf16),
                "negm": negm,
                "ones": np.ones((P, P), dtype=bf16),
            }
        )
    return in_maps


_NC_CACHE = {}


def get_nc(split_waits=True):
    key = ("nc", split_waits)
    if key not in _NC_CACHE:
        _NC_CACHE[key] = build_nc(split_waits)
    return _NC_CACHE[key]


def kernel(x, W_attn, b_attn, W_proj, b_proj):
    in_maps = make_core_inputs(x, W_attn, b_attn, W_proj, b_proj)
    nc = get_nc()
    res = run_bass_kernel_spmd(nc, in_maps, core_ids=list(range(8)))
    parts = [r["out"].astype(np.float32) for r in res.results]
    y = np.empty((B, T, C), dtype=np.float32)
    bpf = np.asarray(b_proj, dtype=np.float32)
    for b in range(B):
        y[b] = parts[4 * b] + parts[4 * b + 1] + parts[4 * b + 2] + parts[4 * b + 3]
        y[b] += bpf
    return y


if __name__ == "__main__":
    rng = np.random.default_rng(0)
    x = rng.standard_normal((B, T, C), dtype=np.float32)
    W_attn = rng.standard_normal((C, 3 * C), dtype=np.float32) / np.sqrt(C)
    b_attn = rng.standard_normal(3 * C).astype(np.float32) * 0.02
    W_proj = rng.standard_normal((C, C), dtype=np.float32) / np.sqrt(C)
    b_proj = rng.standard_normal(C).astype(np.float32) * 0.02
    y = kernel(x, W_attn, b_attn, W_proj, b_proj)
    print(y.shape, y.dtype, float(np.abs(y).mean()))
